# revision 7
# baseline (speedup 1.0000x reference)
"""Trainium2 Bass kernel for nn_Decoder (GRU + Bahdanau attention + fc decoder).

Reference computation (B=64, POI=5000, EMB=256, UNITS=512, QDIM=256):
    x1       = concat(emb[x], query)                   [B, 512]
    output_  = GRUCell(x1, dec_hidden)                 [B, 512]   (keras, reset_after)
    v_proj   = emb @ W1_w + W1_b                       [POI, 512]
    q_proj   = output_ @ W2_w + W2_b                   [B, 512]
    score    = tanh(v_proj[None] + q_proj[:,None]) @ V_w (+V_b)   [B, POI]
    attn     = softmax(score, axis=1)
    context  = sum(attn * emb, axis=1)                 [B, EMB]
    logits   = concat(context, output_, cat_dec_hidden[0]) @ fc_w + fc_b
    returns (logits, state, output_)  with state == output_ == GRU h_new

Sharding over 8 cores:
  - GRU: column-sharded over UNITS (64 cols/core, z/r/h gate-aligned), then
    AllGather of h_new^T (tiny, [64,64] -> [512,64]).
  - v_proj^T: computed REPLICATED on every core (PE is idle during the DMA
    front; this avoids a 27us AllGather of the result).
  - attention tanh/score/softmax/context: BATCH-sharded (8 rows of B per
    core; selected via a one-hot `sel` input so the SPMD program is
    rank-agnostic). Softmax normalizer stays core-local (full POI per row).
  - context: AllGather [8,256] -> [64,256].
  - fc: POI-column-sharded ([1280, 625] per core); host concatenates logits.

V_b is omitted: softmax is shift-invariant so it cannot affect any output.
A_hat is unused by the reference.

Numerics: fp32 except (a) v_proj^T storage + tanh outputs (bf16, feeding the
score dot with bf16 V_w), (b) exp-scores + emb on the context matmul (bf16),
(c) fc weights and its lhsT operand (bf16). PSUM accumulation, GRU, softmax
normalizer, biases and outputs are fp32.
"""
import os
import sys

sys.path.insert(0, "/opt/trn_rl_repo")
os.environ.setdefault("MYCRO_LOCAL_CACHE", "1")

from contextlib import ExitStack

import numpy as np
import ml_dtypes

import concourse.bass as bass
import concourse.tile as tile
from concourse import bacc, mybir
from concourse.bass_utils import run_bass_kernel_spmd
from concourse.masks import make_identity

F32 = mybir.dt.float32
BF16 = mybir.dt.bfloat16
I32 = mybir.dt.int32
AF = mybir.ActivationFunctionType
ALU = mybir.AluOpType

NCORES = 8
POI, EMB, U, QDIM, B = 5000, 256, 512, 256, 64
PS = POI // NCORES          # 625   poi shard (fc columns)
BS = B // NCORES            # 8     batch shard
US = U // NCORES            # 64    units shard (GRU columns)
KIN = EMB + QDIM            # 512   GRU input dim
FCK = EMB + 2 * U           # 1280  fc contraction dim
PHL = POI // 2              # 2500  p-half (exp / transpose granularity)
# N-chunking of the full POI row for score matmuls (PSUM bank = 512 fp32)
SC_CH = [(j * 512, 512) for j in range(9)] + [(4608, POI - 4608)]
# N-chunking of the fc output shard
FC_CH = [(0, 512), (512, PS - 512)]

_CACHE = {}


def _build():
    nc = bacc.Bacc("TRN2", target_bir_lowering=False, debug=False,
                   num_devices=NCORES)

    def din(name, shape, dt=F32):
        return nc.dram_tensor(name, shape, dt, kind="ExternalInput").ap()

    emb_full = din("emb_full", [POI, EMB])          # gather source (fp32)
    emb_bf = din("emb_bf", [POI, EMB], BF16)        # context matmul rhs
    embT_f = din("embT_f", [EMB, POI])              # emb^T, for v_proj rhs
    x_idx = din("x_idx", [B, 1], I32)
    queryT = din("queryT", [QDIM, B])
    dec_hT = din("dec_hT", [U, B])
    dec_hT_s = din("dec_hT_s", [US, B])
    cat_hT_bf = din("cat_hT_bf", [U, B], BF16)
    gru_kT_s = din("gru_kT_s", [KIN, 3 * US])
    gru_rT_s = din("gru_rT_s", [U, 3 * US])
    gru_b0 = din("gru_b0", [3 * US, 1])
    gru_b1 = din("gru_b1", [3 * US, 1])
    w1 = din("w1", [EMB, U])
    w1b = din("w1b", [U, 1])
    w2 = din("w2", [U, U])
    w2b = din("w2b", [U, 1])
    vw = din("vw", [U, 1])
    fcw_s_bf = din("fcw_s_bf", [FCK, PS], BF16)
    fcb_s = din("fcb_s", [1, PS])
    sel = din("sel", [B, BS])

    logits_s = nc.dram_tensor("logits_s", [B, PS], F32, kind="ExternalOutput").ap()
    h_out = nc.dram_tensor("h_out", [B, U], F32, kind="ExternalOutput").ap()

    with tile.TileContext(nc) as tc, ExitStack() as ctx:
        sb = ctx.enter_context(tc.tile_pool(name="sb", bufs=1))
        tp = ctx.enter_context(tc.tile_pool(name="tpool", bufs=5))
        pp = ctx.enter_context(tc.tile_pool(name="pp", bufs=6, space="PSUM"))
        dr = ctx.enter_context(tc.tile_pool(name="dr", bufs=1, space="DRAM"))

        # ---- constants / ACT table prime ---------------------------------
        ident = sb.tile([128, 128], F32)
        make_identity(nc, ident[:])
        prime = sb.tile([1, 8], F32)
        nc.vector.memset(prime[:], 0.0)
        prime2 = sb.tile([1, 8], F32)
        # force the exp_and_others ACT table (has exp AND tanh) to load early
        nc.scalar.activation(prime2[:], prime[:], AF.Exp)
        ones1 = sb.tile([1, B], F32)
        nc.vector.memset(ones1[:], 1.0)

        # ---- input DMAs, critical-path order -----------------------------
        # sync queue order matters: the GRU / gather / v_proj path first,
        # big late-use weights (fcw, emb for context) last.
        def load_chunked(name, src, rows, cols, dt=F32):
            """DRAM [rows, cols] -> SBUF [128, (rows/128)*cols], k-chunk major."""
            nch = rows // 128
            t = sb.tile([128, nch * cols], dt, name=name)
            for k in range(nch):
                nc.sync.dma_start(out=t[:, k * cols:(k + 1) * cols],
                                  in_=src[k * 128:(k + 1) * 128, :])
            return t

        x_sb = sb.tile([B, 1], I32)
        nc.sync.dma_start(out=x_sb[:], in_=x_idx[:])
        b0_sb = sb.tile([US, 3], F32)
        b1_sb = sb.tile([US, 3], F32)
        for g in range(3):
            nc.sync.dma_start(out=b0_sb[:, g:g + 1], in_=gru_b0[g * US:(g + 1) * US, :])
            nc.sync.dma_start(out=b1_sb[:, g:g + 1], in_=gru_b1[g * US:(g + 1) * US, :])
        dhTs_sb = sb.tile([US, B], F32)
        nc.sync.dma_start(out=dhTs_sb[:], in_=dec_hT_s[:])
        sel_sb = sb.tile([B, BS], F32)
        nc.sync.dma_start(out=sel_sb[:], in_=sel[:])
        gk_sb = load_chunked("gk_sb", gru_kT_s, KIN, 3 * US)
        gr_sb = load_chunked("gr_sb", gru_rT_s, U, 3 * US)
        qT_sb = load_chunked("qT_sb", queryT, QDIM, B)
        dhT_sb = load_chunked("dhT_sb", dec_hT, U, B)
        w2_sb = load_chunked("w2_sb", w2, U, U)
        w1_sb = load_chunked("w1_sb", w1, EMB, U)
        w1b_sb = sb.tile([128, 4], F32)
        w2b_sb = sb.tile([128, 4], F32)
        vw_sb = sb.tile([128, 4], F32)
        for k in range(4):
            nc.sync.dma_start(out=w1b_sb[:, k:k + 1], in_=w1b[k * 128:(k + 1) * 128, :])
            nc.sync.dma_start(out=w2b_sb[:, k:k + 1], in_=w2b[k * 128:(k + 1) * 128, :])
            nc.sync.dma_start(out=vw_sb[:, k:k + 1], in_=vw[k * 128:(k + 1) * 128, :])
        vw_bf = sb.tile([128, 4], BF16)
        nc.vector.tensor_copy(vw_bf[:], vw_sb[:])
        chT_sb = load_chunked("chT_sb", cat_hT_bf, U, B, dt=BF16)
        fcb_sb = sb.tile([1, PS], F32)
        nc.sync.dma_start(out=fcb_sb[:], in_=fcb_s[:])
        fcw_sb = load_chunked("fcw_sb", fcw_s_bf, FCK, PS, dt=BF16)
        emb_sb = load_chunked("emb_sb", emb_bf, POI - 8, EMB, dt=BF16)  # 39 chunks
        emb_tail = sb.tile([8, EMB], BF16)
        nc.sync.dma_start(out=emb_tail[:], in_=emb_bf[POI - 8:POI, :])

        # ---- GRU (units-column shard, transposed layout [u_shard, b]) ----
        embx = sb.tile([B, EMB], F32)
        nc.gpsimd.indirect_dma_start(
            out=embx[:], out_offset=None, in_=emb_full[:],
            in_offset=bass.IndirectOffsetOnAxis(ap=x_sb[:, 0:1], axis=0))

        x1T_sb = sb.tile([128, 4 * B], F32)  # x1^T = [emb[x];query]^T, 4 k-chunks
        for c in range(2):
            tpx = pp.tile([128, B], F32, tag="p", name="tpx")
            nc.tensor.transpose(out=tpx[:], in_=embx[:, c * 128:(c + 1) * 128],
                                identity=ident[:B, :B])
            nc.vector.tensor_copy(x1T_sb[:, c * B:(c + 1) * B], tpx[:])
        nc.vector.tensor_copy(x1T_sb[:, 2 * B:3 * B], qT_sb[:, 0:B])
        nc.vector.tensor_copy(x1T_sb[:, 3 * B:4 * B], qT_sb[:, B:2 * B])

        def gate_slice(t, k, g):
            base = k * 3 * US + g * US
            return t[:, base:base + US]

        def gate_psum(g, rec_too):
            ps = pp.tile([US, B], F32, tag="p", name=f"g{g}")
            nmm = 8 if rec_too else 4
            i = 0
            for k in range(4):  # kernel part, K chunks of x1T
                nc.tensor.matmul(ps[:], gate_slice(gk_sb, k, g),
                                 x1T_sb[:, k * B:(k + 1) * B],
                                 start=(i == 0), stop=(i == nmm - 1))
                i += 1
            if rec_too:
                for k in range(4):
                    nc.tensor.matmul(ps[:], gate_slice(gr_sb, k, g),
                                     dhT_sb[:, k * B:(k + 1) * B],
                                     start=False, stop=(i == nmm - 1))
                    i += 1
            return ps

        ps_z = gate_psum(0, True)
        ps_r = gate_psum(1, True)
        ps_xh = gate_psum(2, False)
        ps_hh = pp.tile([US, B], F32, tag="p", name="ps_hh")
        for k in range(4):
            nc.tensor.matmul(ps_hh[:], gate_slice(gr_sb, k, 2),
                             dhT_sb[:, k * B:(k + 1) * B], start=(k == 0), stop=(k == 3))

        # combined half-biases for z, r (sigmoid(x) = 0.5 + 0.5*tanh(x/2))
        bzr = sb.tile([US, 2], F32)
        nc.vector.tensor_add(bzr[:], b0_sb[:, 0:2], b1_sb[:, 0:2])
        bzr_h = sb.tile([US, 2], F32)
        nc.vector.tensor_scalar_mul(bzr_h[:], bzr[:], 0.5)

        zt = sb.tile([US, B], F32)
        nc.scalar.activation(zt[:], ps_z[:], AF.Tanh, bias=bzr_h[:, 0:1], scale=0.5)
        rt = sb.tile([US, B], F32)
        nc.scalar.activation(rt[:], ps_r[:], AF.Tanh, bias=bzr_h[:, 1:2], scale=0.5)

        hh = sb.tile([US, B], F32)
        nc.scalar.add(hh[:], ps_hh[:], b1_sb[:, 2:3])
        m1 = sb.tile([US, B], F32)
        nc.vector.tensor_mul(m1[:], rt[:], hh[:])
        m2 = sb.tile([US, B], F32)
        nc.vector.tensor_add(m2[:], hh[:], m1[:])
        # c_pre = xh + 0.5*(hh + rt*hh)   (r*hh with r = 0.5+0.5*rt)
        cpre = sb.tile([US, B], F32)
        nc.vector.scalar_tensor_tensor(cpre[:], in0=m2[:], scalar=0.5,
                                       in1=ps_xh[:], op0=ALU.mult, op1=ALU.add)
        ct = sb.tile([US, B], F32)
        nc.scalar.activation(ct[:], cpre[:], AF.Tanh, bias=b0_sb[:, 2:3], scale=1.0)
        # h_new = 0.5*((h_prev + c) + zt*(h_prev - c))
        s_ = sb.tile([US, B], F32)
        nc.vector.tensor_add(s_[:], dhTs_sb[:], ct[:])
        d_ = sb.tile([US, B], F32)
        nc.vector.tensor_sub(d_[:], dhTs_sb[:], ct[:])
        m_ = sb.tile([US, B], F32)
        nc.vector.tensor_mul(m_[:], zt[:], d_[:])
        hn2 = sb.tile([US, B], F32)
        nc.vector.tensor_add(hn2[:], s_[:], m_[:])
        hnT_s = sb.tile([US, B], F32)
        nc.vector.tensor_scalar_mul(hnT_s[:], hn2[:], 0.5)

        # AllGather h_new^T shards -> full h^T [512, 64]
        ag1_in = dr.tile([US, B], F32)
        ag1_out = dr.tile([U, B], F32, addr_space="Shared")
        nc.sync.dma_start(out=ag1_in[:], in_=hnT_s[:])
        nc.gpsimd.collective_compute(
            "AllGather", ALU.bypass, replica_groups=[list(range(NCORES))],
            ins=[ag1_in[:]], outs=[ag1_out[:]])
        hT_sb = sb.tile([128, 4 * B], F32)
        for k in range(4):
            nc.sync.dma_start(out=hT_sb[:, k * B:(k + 1) * B],
                              in_=ag1_out[k * 128:(k + 1) * 128, :])
        hT_bf = sb.tile([128, 4 * B], BF16)   # bf16 copy for the fc lhsT
        nc.vector.tensor_copy(hT_bf[:], hT_sb[:])

        # h (non-transposed) for the state output + sel matmul
        h_sb = sb.tile([B, U], F32)
        for k in range(4):
            tph = pp.tile([B, 128], F32, tag="p", name="tph")
            nc.tensor.transpose(out=tph[:], in_=hT_sb[:, k * B:(k + 1) * B],
                                identity=ident[:, :])
            nc.vector.tensor_copy(h_sb[:, k * 128:(k + 1) * 128], tph[:])
        nc.sync.dma_start(out=h_out[:], in_=h_sb[:])

        # my 8 batch rows of h via one-hot selection (keeps program rank-agnostic)
        ps_hm = pp.tile([BS, U], F32, tag="p", name="ps_hm")
        nc.tensor.matmul(ps_hm[:], sel_sb[:], h_sb[:], start=True, stop=True)
        hm_sb = sb.tile([BS, U], F32)
        nc.vector.tensor_copy(hm_sb[:], ps_hm[:])
        hTm_sb = sb.tile([128, 4 * BS], F32)
        for k in range(4):
            tpm = pp.tile([128, BS], F32, tag="p", name="tpm")
            nc.tensor.transpose(out=tpm[:], in_=hm_sb[:, k * 128:(k + 1) * 128],
                                identity=ident[:BS, :BS])
            nc.vector.tensor_copy(hTm_sb[:, k * BS:(k + 1) * BS], tpm[:])

        # biasT[u, j] = q_proj^T + W1_b + W2_b   for my 8 batch rows
        bv_sb = sb.tile([128, 4], F32)
        nc.vector.tensor_add(bv_sb[:], w1b_sb[:], w2b_sb[:])
        biasT = sb.tile([128, 4 * BS], F32)
        for m in range(4):
            ps_q = pp.tile([128, BS], F32, tag="p", name="ps_q")
            for k in range(4):
                nc.tensor.matmul(ps_q[:], w2_sb[:, k * U + m * 128:k * U + (m + 1) * 128],
                                 hTm_sb[:, k * BS:(k + 1) * BS],
                                 start=(k == 0), stop=(k == 3))
            nc.scalar.add(biasT[:, m * BS:(m + 1) * BS], ps_q[:], bv_sb[:, m:m + 1])

        # ---- v_proj^T computed fully on every core (bf16 result) ---------
        # embT streamed from DRAM per n-block (keeping all 5 MB resident
        # would blow the SBUF budget); loop n-outer, m-inner so each loaded
        # block feeds all 4 m-chunks.
        vfull = sb.tile([128, 4 * POI], BF16)
        for (n0, nl) in SC_CH:
            ets = []
            for k in range(2):
                et = tp.tile([128, 512], F32, name="et", bufs=4)
                nc.sync.dma_start(out=et[:, :nl],
                                  in_=embT_f[k * 128:(k + 1) * 128, n0:n0 + nl])
                ets.append(et)
            for m in range(4):
                ps_v = pp.tile([128, 512], F32, tag="p", name="ps_v")
                for k in range(2):
                    nc.tensor.matmul(ps_v[:, :nl],
                                     w1_sb[:, k * U + m * 128:k * U + (m + 1) * 128],
                                     ets[k][:, :nl],
                                     start=(k == 0), stop=(k == 1))
                nc.vector.tensor_copy(vfull[:, m * POI + n0:m * POI + n0 + nl],
                                      ps_v[:, :nl])

        # ---- attention main loop (ACT-bound) -----------------------------
        # tanh in [128, 5000] tiles: 4 ACT instructions per batch row.
        # Score matmuls in two waves of 5 PSUM chunks (PSUM has 8 banks).
        scores = sb.tile([BS, POI], F32)
        n_ph = sb.tile([BS, 2], F32)
        for bi in range(BS):
            t_tiles = []
            scs_a = [pp.tile([1, 512], F32, tag="p", name=f"sa{j}")
                     for j in range(5)]
            for m in range(4):
                t_bf = tp.tile([128, POI], BF16, tag="T", name="t_bf")
                t_tiles.append(t_bf)
                nc.scalar.activation(
                    t_bf[:], vfull[:, m * POI:(m + 1) * POI],
                    AF.Tanh, bias=biasT[:, m * BS + bi:m * BS + bi + 1])
                for j in range(5):
                    n0, nl = SC_CH[j]
                    nc.tensor.matmul(scs_a[j][:, :nl], vw_bf[:, m:m + 1],
                                     t_bf[:, n0:n0 + nl],
                                     start=(m == 0), stop=(m == 3))
            scs_b = [pp.tile([1, 512], F32, tag="p", name=f"sb{j}")
                     for j in range(5)]
            for m in range(4):
                for j in range(5):
                    n0, nl = SC_CH[5 + j]
                    nc.tensor.matmul(scs_b[j][:, :nl], vw_bf[:, m:m + 1],
                                     t_tiles[m][:, n0:n0 + nl],
                                     start=(m == 0), stop=(m == 3))
            # evacuate the 10 chunks to a partition-0 staging row, then DMA
            # into scores[bi, :] (engines cannot write at base partition != 0)
            q = nc.sync if bi % 2 == 0 else nc.gpsimd
            for j, (n0, nl) in enumerate(SC_CH):
                st = tp.tile([1, 512], F32, name="st", bufs=4)
                src = (scs_a if j < 5 else scs_b)[j % 5]
                nc.vector.tensor_copy(st[:1, :nl], src[:1, :nl])
                q.dma_start(out=scores[bi:bi + 1, n0:n0 + nl], in_=st[:1, :nl])

        # exp in place per p-half; accumulate the softmax normalizer
        expT = sb.tile([128, 40 * BS], BF16)
        for ph in range(2):
            nc.scalar.activation(scores[:, ph * PHL:(ph + 1) * PHL],
                                 scores[:, ph * PHL:(ph + 1) * PHL],
                                 AF.Exp, accum_out=n_ph[:, ph:ph + 1])
            # transpose this half's exp scores into [128, 8] chunks (bf16)
            for c in range(20 * ph, 20 * (ph + 1)):
                w = 128 if c < 39 else POI - 39 * 128  # 8-wide tail chunk
                tpe = pp.tile([128, BS], F32, tag="p", name="tpe")
                nc.tensor.transpose(out=tpe[:w, :],
                                    in_=scores[:, c * 128:c * 128 + w],
                                    identity=ident[:BS, :BS])
                nc.vector.tensor_copy(expT[:w, c * BS:(c + 1) * BS], tpe[:w, :])

        n_sb = sb.tile([BS, 1], F32)
        nc.vector.tensor_add(n_sb[:], n_ph[:, 0:1], n_ph[:, 1:2])
        rn_sb = sb.tile([BS, 1], F32)
        nc.vector.reciprocal(rn_sb[:], n_sb[:])

        # context (unnormalized): ctx[j, e] = sum_p exp[j, p] * emb[p, e]
        ps_ctx = pp.tile([BS, EMB], F32, tag="p", name="ps_ctx")
        for c in range(39):
            nc.tensor.matmul(ps_ctx[:], expT[:, c * BS:(c + 1) * BS],
                             emb_sb[:, c * EMB:(c + 1) * EMB],
                             start=(c == 0), stop=False)
        nc.tensor.matmul(ps_ctx[:], expT[:8, 39 * BS:40 * BS], emb_tail[:],
                         start=False, stop=True)
        ctx_sb = sb.tile([BS, EMB], F32)
        nc.vector.tensor_scalar_mul(ctx_sb[:], ps_ctx[:], rn_sb[:, 0:1])

        ag3_in = dr.tile([BS, EMB], F32)
        ag3_out = dr.tile([B, EMB], F32, addr_space="Shared")
        nc.sync.dma_start(out=ag3_in[:], in_=ctx_sb[:])
        nc.gpsimd.collective_compute(
            "AllGather", ALU.bypass, replica_groups=[list(range(NCORES))],
            ins=[ag3_in[:]], outs=[ag3_out[:]])
        ctxf = sb.tile([B, EMB], F32)
        nc.sync.dma_start(out=ctxf[:], in_=ag3_out[:])

        ctxT = sb.tile([128, 2 * B], BF16)
        for k in range(2):
            tpc = pp.tile([128, B], F32, tag="p", name="tpc")
            nc.tensor.transpose(out=tpc[:], in_=ctxf[:, k * 128:(k + 1) * 128],
                                identity=ident[:B, :B])
            nc.vector.tensor_copy(ctxT[:, k * B:(k + 1) * B], tpc[:])

        # ---- fc: logits = out_cat @ fc_w + fc_b (poi-column shard) -------
        # k-chunk order: bias, h, cat_hidden first; ctx LAST so the first 9
        # accumulation steps overlap the context AllGather.
        cat_chunks = ([hT_bf[:, k * B:(k + 1) * B] for k in range(4)]
                      + [chT_sb[:, k * B:(k + 1) * B] for k in range(4)]
                      + [ctxT[:, k * B:(k + 1) * B] for k in range(2)])
        krows = [2, 3, 4, 5, 6, 7, 8, 9, 0, 1]  # fc_w row-chunk per cat_chunk
        for (n0, nl) in FC_CH:
            ps_fc = pp.tile([B, 512], F32, tag="p", name="ps_fc")
            nc.tensor.matmul(ps_fc[:, :nl], ones1[:, :], fcb_sb[:, n0:n0 + nl],
                             start=True, stop=False)
            for i, kr in enumerate(krows):
                nc.tensor.matmul(ps_fc[:, :nl], cat_chunks[i],
                                 fcw_sb[:, kr * PS + n0:kr * PS + n0 + nl],
                                 start=False, stop=(i == 9))
            lg = sb.tile([B, 512], F32, name="lg")
            nc.vector.tensor_copy(lg[:, :nl], ps_fc[:, :nl])
            nc.sync.dma_start(out=logits_s[:, n0:n0 + nl], in_=lg[:, :nl])

    nc.compile()
    return nc


def _prep_inputs(inputs):
    """Host-side sharding / layout prep (data movement + dtype casts only)."""
    f = lambda a: np.ascontiguousarray(np.asarray(a), dtype=np.float32)
    emb = f(inputs["poi_embedding"])
    query = f(inputs["query"])
    dec_h = f(inputs["dec_hidden"])
    cat_h = f(inputs["cat_dec_hidden"])[0]
    gk = f(inputs["gru_kernel"])
    gr = f(inputs["gru_rec_kernel"])
    gb = f(inputs["gru_bias"])
    w1 = f(inputs["W1_w"]); w1b = f(inputs["W1_b"]).reshape(U, 1)
    w2 = f(inputs["W2_w"]); w2b = f(inputs["W2_b"]).reshape(U, 1)
    vw = f(inputs["V_w"]).reshape(U, 1)
    fcw = f(inputs["fc_w"]); fcb = f(inputs["fc_b"]).reshape(1, POI)
    x = np.asarray(inputs["x"]).astype(np.int32).reshape(B, 1)

    emb_bf = emb.astype(ml_dtypes.bfloat16)
    embT = np.ascontiguousarray(emb.T)
    queryT = np.ascontiguousarray(query.T)
    dec_hT = np.ascontiguousarray(dec_h.T)
    cat_hT_bf = np.ascontiguousarray(cat_h.T).astype(ml_dtypes.bfloat16)

    in_maps = []
    for c in range(NCORES):
        cols = np.r_[c * US:(c + 1) * US,
                     U + c * US:U + (c + 1) * US,
                     2 * U + c * US:2 * U + (c + 1) * US]
        selm = np.zeros((B, BS), np.float32)
        selm[c * BS + np.arange(BS), np.arange(BS)] = 1.0
        in_maps.append({
            "emb_full": emb,
            "emb_bf": emb_bf,
            "embT_f": embT,
            "x_idx": x,
            "queryT": queryT,
            "dec_hT": dec_hT,
            "dec_hT_s": np.ascontiguousarray(dec_hT[c * US:(c + 1) * US, :]),
            "cat_hT_bf": cat_hT_bf,
            "gru_kT_s": np.ascontiguousarray(gk[:, cols]),
            "gru_rT_s": np.ascontiguousarray(gr[:, cols]),
            "gru_b0": np.ascontiguousarray(gb[0, cols]).reshape(3 * US, 1),
            "gru_b1": np.ascontiguousarray(gb[1, cols]).reshape(3 * US, 1),
            "w1": w1, "w1b": w1b, "w2": w2, "w2b": w2b, "vw": vw,
            "fcw_s_bf": np.ascontiguousarray(fcw[:, c * PS:(c + 1) * PS]).astype(ml_dtypes.bfloat16),
            "fcb_s": np.ascontiguousarray(fcb[:, c * PS:(c + 1) * PS]),
            "sel": selm,
        })
    return in_maps


def kernel(**inputs):
    if "nc" not in _CACHE:
        _CACHE["nc"] = _build()
    nc = _CACHE["nc"]
    in_maps = _prep_inputs(inputs)
    res = run_bass_kernel_spmd(nc, in_maps, list(range(NCORES)))
    r = res.results
    logits = np.concatenate([r[c]["logits_s"] for c in range(NCORES)], axis=1)
    h_new = r[0]["h_out"]
    return (logits, h_new, h_new.copy())


# revision 9
# speedup vs baseline: 1.0707x; 1.0707x over previous
"""Trainium2 Bass kernel for nn_Decoder (GRU + Bahdanau attention + fc decoder).

Reference computation (B=64, POI=5000, EMB=256, UNITS=512, QDIM=256):
    x1       = concat(emb[x], query)                   [B, 512]
    output_  = GRUCell(x1, dec_hidden)                 [B, 512]   (keras, reset_after)
    v_proj   = emb @ W1_w + W1_b                       [POI, 512]
    q_proj   = output_ @ W2_w + W2_b                   [B, 512]
    score    = tanh(v_proj[None] + q_proj[:,None]) @ V_w (+V_b)   [B, POI]
    attn     = softmax(score, axis=1)
    context  = sum(attn * emb, axis=1)                 [B, EMB]
    logits   = concat(context, output_, cat_dec_hidden[0]) @ fc_w + fc_b
    returns (logits, state, output_)  with state == output_ == GRU h_new

Sharding over 8 cores:
  - GRU: column-sharded over UNITS (64 cols/core, z/r/h gate-aligned), then
    AllGather of h_new^T (tiny, [64,64] -> [512,64]).
  - v_proj^T: computed REPLICATED on every core (PE is idle during the DMA
    front; avoids a 27us AllGather of the result).
  - attention tanh/score/softmax/context: BATCH-sharded (8 rows of B per
    core; selected via a one-hot `sel` input so the SPMD program is
    rank-agnostic). Softmax normalizer stays core-local (full POI per row).
  - context: AllGather [8,256] -> [64,256].
  - fc: POI-column-sharded ([1280, 625] per core); host concatenates logits.

V_b is omitted: softmax is shift-invariant so it cannot affect any output.
A_hat is unused by the reference.

Numerics: fp32 except (a) v_proj^T storage + tanh outputs (bf16, feeding the
score dot with bf16 V_w), (b) exp-scores + emb on the context matmul (bf16),
(c) fc weights and its lhsT operand (bf16). PSUM accumulation, GRU, softmax
normalizer, biases and outputs are fp32.
"""
import os
import sys

sys.path.insert(0, "/opt/trn_rl_repo")
os.environ.setdefault("MYCRO_LOCAL_CACHE", "1")

from contextlib import ExitStack

import numpy as np
import ml_dtypes

import concourse.bass as bass
import concourse.tile as tile
from concourse import bacc, mybir
from concourse.bass_utils import run_bass_kernel_spmd
from concourse.masks import make_identity

F32 = mybir.dt.float32
BF16 = mybir.dt.bfloat16
I32 = mybir.dt.int32
AF = mybir.ActivationFunctionType
ALU = mybir.AluOpType

NCORES = 8
POI, EMB, U, QDIM, B = 5000, 256, 512, 256, 64
PS = POI // NCORES          # 625   poi shard (fc columns)
BS = B // NCORES            # 8     batch shard
US = U // NCORES            # 64    units shard (GRU columns)
KIN = EMB + QDIM            # 512   GRU input dim
FCK = EMB + 2 * U           # 1280  fc contraction dim
PHL = POI // 2              # 2500  p-half (exp / transpose granularity)
# N-chunking of the full POI row for score matmuls (PSUM bank = 512 fp32)
SC_CH = [(j * 512, 512) for j in range(9)] + [(4608, POI - 4608)]
# N-chunking of the fc output shard
FC_CH = [(0, 512), (512, PS - 512)]

_CACHE = {}


def _build():
    nc = bacc.Bacc("TRN2", target_bir_lowering=False, debug=False,
                   num_devices=NCORES)

    def din(name, shape, dt=F32):
        return nc.dram_tensor(name, shape, dt, kind="ExternalInput").ap()

    emb_full = din("emb_full", [POI, EMB])          # gather source (fp32)
    emb_bf = din("emb_bf", [POI, EMB], BF16)        # context matmul rhs
    embT_f = din("embT_f", [EMB, POI])              # emb^T, for v_proj rhs
    x_idx = din("x_idx", [B, 1], I32)
    queryT = din("queryT", [QDIM, B])
    dec_hT = din("dec_hT", [U, B])
    dec_hT_s = din("dec_hT_s", [US, B])
    cat_hT_bf = din("cat_hT_bf", [U, B], BF16)
    gru_kT_s = din("gru_kT_s", [KIN, 3 * US])
    gru_rT_s = din("gru_rT_s", [U, 3 * US])
    gru_b0 = din("gru_b0", [3 * US, 1])
    gru_b1 = din("gru_b1", [3 * US, 1])
    w1 = din("w1", [EMB, U])
    w1b = din("w1b", [U, 1])
    w2 = din("w2", [U, U])
    w2b = din("w2b", [U, 1])
    vw = din("vw", [U, 1])
    fcw_s_bf = din("fcw_s_bf", [FCK, PS], BF16)
    fcb_s = din("fcb_s", [1, PS])
    sel = din("sel", [B, BS])

    logits_s = nc.dram_tensor("logits_s", [B, PS], F32, kind="ExternalOutput").ap()
    h_out = nc.dram_tensor("h_out", [B, U], F32, kind="ExternalOutput").ap()

    with tile.TileContext(nc) as tc, ExitStack() as ctx:
        sb = ctx.enter_context(tc.tile_pool(name="sb", bufs=1))
        tp = ctx.enter_context(tc.tile_pool(name="tpool", bufs=6))
        pp = ctx.enter_context(tc.tile_pool(name="pp", bufs=6, space="PSUM"))
        dr = ctx.enter_context(tc.tile_pool(name="dr", bufs=1, space="DRAM"))

        # ---- constants / ACT table prime ---------------------------------
        ident = sb.tile([128, 128], F32)
        make_identity(nc, ident[:])
        prime = sb.tile([1, 8], F32)
        nc.vector.memset(prime[:], 0.0)
        prime2 = sb.tile([1, 8], F32)
        # force the exp_and_others ACT table (has exp AND tanh) to load early
        nc.scalar.activation(prime2[:], prime[:], AF.Exp)
        ones1 = sb.tile([1, B], F32)
        nc.vector.memset(ones1[:], 1.0)

        # ---- input DMAs, critical-path order, one DMA per tensor ---------
        def load_chunked(name, src, rows, cols, dt=F32, q=None):
            """DRAM [rows, cols] -> SBUF [128, (rows/128)*cols] in one DMA."""
            nch = rows // 128
            t = sb.tile([128, nch * cols], dt, name=name)
            (q or nc.sync).dma_start(
                out=t[:].rearrange("p (k c) -> p k c", k=nch),
                in_=src.rearrange("(k p) c -> p k c", p=128))
            return t

        x_sb = sb.tile([B, 1], I32)
        nc.sync.dma_start(out=x_sb[:], in_=x_idx[:])
        b0_sb = sb.tile([US, 3], F32)
        nc.sync.dma_start(out=b0_sb[:].rearrange("p (g o) -> p g o", g=3),
                          in_=gru_b0.rearrange("(g p) o -> p g o", p=US))
        b1_sb = sb.tile([US, 3], F32)
        nc.sync.dma_start(out=b1_sb[:].rearrange("p (g o) -> p g o", g=3),
                          in_=gru_b1.rearrange("(g p) o -> p g o", p=US))
        dhTs_sb = sb.tile([US, B], F32)
        nc.sync.dma_start(out=dhTs_sb[:], in_=dec_hT_s[:])
        sel_sb = sb.tile([B, BS], F32)
        nc.sync.dma_start(out=sel_sb[:], in_=sel[:])
        gk_sb = load_chunked("gk_sb", gru_kT_s, KIN, 3 * US)
        gr_sb = load_chunked("gr_sb", gru_rT_s, U, 3 * US)
        qT_sb = load_chunked("qT_sb", queryT, QDIM, B)
        dhT_sb = load_chunked("dhT_sb", dec_hT, U, B)
        w1_sb = load_chunked("w1_sb", w1, EMB, U)
        # embT streamed per 512-col block for v_proj (both k-chunks per DMA)
        et_tiles = []
        for (n0, nl) in SC_CH:
            et = tp.tile([128, 2 * 512], F32, name="et", bufs=3)
            nc.sync.dma_start(
                out=et[:, :2 * nl].rearrange("p (k c) -> p k c", k=2),
                in_=embT_f[:, n0:n0 + nl].rearrange("(k p) c -> p k c", p=128))
            et_tiles.append(et)
        w2_sb = load_chunked("w2_sb", w2, U, U)
        wbv_sb = sb.tile([128, 12], F32)
        for i, t in enumerate((w1b, w2b, vw)):
            nc.sync.dma_start(
                out=wbv_sb[:, 4 * i:4 * (i + 1)].rearrange("p (k o) -> p k o", k=4),
                in_=t.rearrange("(k p) o -> p k o", p=128))
        w1b_sb, w2b_sb, vw_sb = wbv_sb[:, 0:4], wbv_sb[:, 4:8], wbv_sb[:, 8:12]
        vw_bf = sb.tile([128, 4], BF16)
        nc.vector.tensor_copy(vw_bf[:], vw_sb)
        chT_sb = load_chunked("chT_sb", cat_hT_bf, U, B, dt=BF16)
        fcb_sb = sb.tile([1, PS], F32)
        nc.sync.dma_start(out=fcb_sb[:], in_=fcb_s[:])
        fcw_sb = load_chunked("fcw_sb", fcw_s_bf, FCK, PS, dt=BF16)
        emb_sb = load_chunked("emb_sb", emb_bf[0:POI - 8, :], POI - 8, EMB, dt=BF16)
        emb_tail = sb.tile([8, EMB], BF16)
        nc.sync.dma_start(out=emb_tail[:], in_=emb_bf[POI - 8:POI, :])

        # ---- GRU (units-column shard, transposed layout [u_shard, b]) ----
        embx = sb.tile([B, EMB], F32)
        nc.gpsimd.indirect_dma_start(
            out=embx[:], out_offset=None, in_=emb_full[:],
            in_offset=bass.IndirectOffsetOnAxis(ap=x_sb[:, 0:1], axis=0))

        x1T_sb = sb.tile([128, 4 * B], F32)  # x1^T = [emb[x];query]^T, 4 k-chunks
        for c in range(2):
            tpx = pp.tile([128, B], F32, tag="p", name="tpx")
            nc.tensor.transpose(out=tpx[:], in_=embx[:, c * 128:(c + 1) * 128],
                                identity=ident[:B, :B])
            nc.vector.tensor_copy(x1T_sb[:, c * B:(c + 1) * B], tpx[:])
        nc.vector.tensor_copy(x1T_sb[:, 2 * B:4 * B], qT_sb[:, 0:2 * B])

        def gate_slice(t, k, g):
            base = k * 3 * US + g * US
            return t[:, base:base + US]

        def gate_psum(g, rec_too):
            ps = pp.tile([US, B], F32, tag="p", name=f"g{g}")
            nmm = 8 if rec_too else 4
            i = 0
            for k in range(4):  # kernel part, K chunks of x1T
                nc.tensor.matmul(ps[:], gate_slice(gk_sb, k, g),
                                 x1T_sb[:, k * B:(k + 1) * B],
                                 start=(i == 0), stop=(i == nmm - 1))
                i += 1
            if rec_too:
                for k in range(4):
                    nc.tensor.matmul(ps[:], gate_slice(gr_sb, k, g),
                                     dhT_sb[:, k * B:(k + 1) * B],
                                     start=False, stop=(i == nmm - 1))
                    i += 1
            return ps

        ps_z = gate_psum(0, True)
        ps_r = gate_psum(1, True)
        ps_xh = gate_psum(2, False)
        ps_hh = pp.tile([US, B], F32, tag="p", name="ps_hh")
        for k in range(4):
            nc.tensor.matmul(ps_hh[:], gate_slice(gr_sb, k, 2),
                             dhT_sb[:, k * B:(k + 1) * B], start=(k == 0), stop=(k == 3))

        # combined half-biases for z, r (sigmoid(x) = 0.5 + 0.5*tanh(x/2))
        bzr = sb.tile([US, 2], F32)
        nc.vector.tensor_add(bzr[:], b0_sb[:, 0:2], b1_sb[:, 0:2])
        bzr_h = sb.tile([US, 2], F32)
        nc.vector.tensor_scalar_mul(bzr_h[:], bzr[:], 0.5)

        zt = sb.tile([US, B], F32)
        nc.scalar.activation(zt[:], ps_z[:], AF.Tanh, bias=bzr_h[:, 0:1], scale=0.5)
        rt = sb.tile([US, B], F32)
        nc.scalar.activation(rt[:], ps_r[:], AF.Tanh, bias=bzr_h[:, 1:2], scale=0.5)

        hh = sb.tile([US, B], F32)
        nc.scalar.add(hh[:], ps_hh[:], b1_sb[:, 2:3])
        m1 = sb.tile([US, B], F32)
        nc.vector.tensor_mul(m1[:], rt[:], hh[:])
        m2 = sb.tile([US, B], F32)
        nc.vector.tensor_add(m2[:], hh[:], m1[:])
        # c_pre = xh + 0.5*(hh + rt*hh)   (r*hh with r = 0.5+0.5*rt)
        cpre = sb.tile([US, B], F32)
        nc.vector.scalar_tensor_tensor(cpre[:], in0=m2[:], scalar=0.5,
                                       in1=ps_xh[:], op0=ALU.mult, op1=ALU.add)
        ct = sb.tile([US, B], F32)
        nc.scalar.activation(ct[:], cpre[:], AF.Tanh, bias=b0_sb[:, 2:3], scale=1.0)
        # h_new = 0.5*((h_prev + c) + zt*(h_prev - c))
        s_ = sb.tile([US, B], F32)
        nc.vector.tensor_add(s_[:], dhTs_sb[:], ct[:])
        d_ = sb.tile([US, B], F32)
        nc.vector.tensor_sub(d_[:], dhTs_sb[:], ct[:])
        m_ = sb.tile([US, B], F32)
        nc.vector.tensor_mul(m_[:], zt[:], d_[:])
        hn2 = sb.tile([US, B], F32)
        nc.vector.tensor_add(hn2[:], s_[:], m_[:])
        hnT_s = sb.tile([US, B], F32)
        nc.vector.tensor_scalar_mul(hnT_s[:], hn2[:], 0.5)

        # AllGather h_new^T shards -> full h^T [512, 64]
        ag1_in = dr.tile([US, B], F32)
        ag1_out = dr.tile([U, B], F32, addr_space="Shared")
        nc.sync.dma_start(out=ag1_in[:], in_=hnT_s[:])
        nc.gpsimd.collective_compute(
            "AllGather", ALU.bypass, replica_groups=[list(range(NCORES))],
            ins=[ag1_in[:]], outs=[ag1_out[:]])
        hT_sb = sb.tile([128, 4 * B], F32)
        nc.sync.dma_start(out=hT_sb[:].rearrange("p (k b) -> p k b", k=4),
                          in_=ag1_out.rearrange("(k p) b -> p k b", p=128))
        hT_bf = sb.tile([128, 4 * B], BF16)   # bf16 copy for the fc lhsT
        nc.vector.tensor_copy(hT_bf[:], hT_sb[:])

        # h (non-transposed) for the state output + sel matmul
        h_sb = sb.tile([B, U], F32)
        for k in range(4):
            tph = pp.tile([B, 128], F32, tag="p", name="tph")
            nc.tensor.transpose(out=tph[:], in_=hT_sb[:, k * B:(k + 1) * B],
                                identity=ident[:, :])
            nc.vector.tensor_copy(h_sb[:, k * 128:(k + 1) * 128], tph[:])
        nc.sync.dma_start(out=h_out[:], in_=h_sb[:])

        # my 8 batch rows of h via one-hot selection (keeps program rank-agnostic)
        ps_hm = pp.tile([BS, U], F32, tag="p", name="ps_hm")
        nc.tensor.matmul(ps_hm[:], sel_sb[:], h_sb[:], start=True, stop=True)
        hm_sb = sb.tile([BS, U], F32)
        nc.vector.tensor_copy(hm_sb[:], ps_hm[:])
        hTm_sb = sb.tile([128, 4 * BS], F32)
        for k in range(4):
            tpm = pp.tile([128, BS], F32, tag="p", name="tpm")
            nc.tensor.transpose(out=tpm[:], in_=hm_sb[:, k * 128:(k + 1) * 128],
                                identity=ident[:BS, :BS])
            nc.vector.tensor_copy(hTm_sb[:, k * BS:(k + 1) * BS], tpm[:])

        # biasT[u, j] = q_proj^T + W1_b + W2_b   for my 8 batch rows
        bv_sb = sb.tile([128, 4], F32)
        nc.vector.tensor_add(bv_sb[:], w1b_sb, w2b_sb)
        biasT = sb.tile([128, 4 * BS], F32)
        for m in range(4):
            ps_q = pp.tile([128, BS], F32, tag="p", name="ps_q")
            for k in range(4):
                nc.tensor.matmul(ps_q[:], w2_sb[:, k * U + m * 128:k * U + (m + 1) * 128],
                                 hTm_sb[:, k * BS:(k + 1) * BS],
                                 start=(k == 0), stop=(k == 3))
            nc.scalar.add(biasT[:, m * BS:(m + 1) * BS], ps_q[:], bv_sb[:, m:m + 1])

        # ---- v_proj^T computed fully on every core (bf16 result) ---------
        vfull = sb.tile([128, 4 * POI], BF16)
        for j, (n0, nl) in enumerate(SC_CH):
            for m in range(4):
                ps_v = pp.tile([128, 512], F32, tag="p", name="ps_v")
                for k in range(2):
                    nc.tensor.matmul(ps_v[:, :nl],
                                     w1_sb[:, k * U + m * 128:k * U + (m + 1) * 128],
                                     et_tiles[j][:, k * nl:(k + 1) * nl],
                                     start=(k == 0), stop=(k == 1))
                nc.vector.tensor_copy(vfull[:, m * POI + n0:m * POI + n0 + nl],
                                      ps_v[:, :nl])

        # ---- attention main loop (ACT-bound) -----------------------------
        # tanh in [128, 5000] tiles: 4 ACT instructions per batch row.
        # Wave A: chunks 0-4 accumulate per-tanh (keeps PE fed).
        # Wave B: chunks 5-9 chunk-major after all 4 tanhs (long warm PE
        # burst, early PSUM evac keeps <= 3 score banks live).
        scores = sb.tile([BS, POI], F32)
        n_ph = sb.tile([BS, 2], F32)
        for bi in range(BS):
            t_tiles = []
            scs_a = [pp.tile([1, 512], F32, tag="p", name=f"sa{j}")
                     for j in range(5)]
            for m in range(4):
                t_bf = tp.tile([128, POI], BF16, tag="T", name="t_bf")
                t_tiles.append(t_bf)
                nc.scalar.activation(
                    t_bf[:], vfull[:, m * POI:(m + 1) * POI],
                    AF.Tanh, bias=biasT[:, m * BS + bi:m * BS + bi + 1])
                for j in range(5):
                    n0, nl = SC_CH[j]
                    nc.tensor.matmul(scs_a[j][:, :nl], vw_bf[:, m:m + 1],
                                     t_bf[:, n0:n0 + nl],
                                     start=(m == 0), stop=(m == 3))
            q = nc.sync if bi % 2 == 0 else nc.gpsimd
            for j in range(5):
                n0, nl = SC_CH[j]
                st = tp.tile([1, 512], F32, name="st", bufs=3)
                nc.vector.tensor_copy(st[:1, :nl], scs_a[j][:1, :nl])
                q.dma_start(out=scores[bi:bi + 1, n0:n0 + nl], in_=st[:1, :nl])
            for j in range(5, 10):
                n0, nl = SC_CH[j]
                ps_sc = pp.tile([1, 512], F32, tag="p", name="ps_sc")
                for m in range(4):
                    nc.tensor.matmul(ps_sc[:, :nl], vw_bf[:, m:m + 1],
                                     t_tiles[m][:, n0:n0 + nl],
                                     start=(m == 0), stop=(m == 3))
                st = tp.tile([1, 512], F32, name="st", bufs=3)
                nc.vector.tensor_copy(st[:1, :nl], ps_sc[:1, :nl])
                q.dma_start(out=scores[bi:bi + 1, n0:n0 + nl], in_=st[:1, :nl])

        # exp in place per p-half; accumulate the softmax normalizer
        expT = sb.tile([128, 40 * BS], BF16)
        for ph in range(2):
            nc.scalar.activation(scores[:, ph * PHL:(ph + 1) * PHL],
                                 scores[:, ph * PHL:(ph + 1) * PHL],
                                 AF.Exp, accum_out=n_ph[:, ph:ph + 1])
            # transpose this half's exp scores into [128, 8] chunks (bf16)
            for c in range(20 * ph, 20 * (ph + 1)):
                w = 128 if c < 39 else POI - 39 * 128  # 8-wide tail chunk
                tpe = pp.tile([128, BS], F32, tag="p", name="tpe")
                nc.tensor.transpose(out=tpe[:w, :],
                                    in_=scores[:, c * 128:c * 128 + w],
                                    identity=ident[:BS, :BS])
                nc.vector.tensor_copy(expT[:w, c * BS:(c + 1) * BS], tpe[:w, :])

        n_sb = sb.tile([BS, 1], F32)
        nc.vector.tensor_add(n_sb[:], n_ph[:, 0:1], n_ph[:, 1:2])
        rn_sb = sb.tile([BS, 1], F32)
        nc.vector.reciprocal(rn_sb[:], n_sb[:])

        # context (unnormalized): ctx[j, e] = sum_p exp[j, p] * emb[p, e]
        ps_ctx = pp.tile([BS, EMB], F32, tag="p", name="ps_ctx")
        for c in range(39):
            nc.tensor.matmul(ps_ctx[:], expT[:, c * BS:(c + 1) * BS],
                             emb_sb[:, c * EMB:(c + 1) * EMB],
                             start=(c == 0), stop=False)
        nc.tensor.matmul(ps_ctx[:], expT[:8, 39 * BS:40 * BS], emb_tail[:],
                         start=False, stop=True)
        ctx_sb = sb.tile([BS, EMB], F32)
        nc.vector.tensor_scalar_mul(ctx_sb[:], ps_ctx[:], rn_sb[:, 0:1])

        # ---- fc partials (bias + h + cat chunks) BEFORE the ctx AllGather,
        # so these matmuls execute on PE while the collective runs.
        ps_fcs = []
        for (n0, nl) in FC_CH:
            ps_fc = pp.tile([B, 512], F32, tag="fc", name="ps_fc", bufs=2)
            ps_fcs.append(ps_fc)
            nc.tensor.matmul(ps_fc[:, :nl], ones1[:, :], fcb_sb[:, n0:n0 + nl],
                             start=True, stop=False)
            for i in range(8):   # h chunks (fc rows 256:768), cat (768:1280)
                kr = 2 + i
                src = hT_bf if i < 4 else chT_sb
                lhsT = src[:, (i % 4) * B:(i % 4 + 1) * B]
                nc.tensor.matmul(ps_fc[:, :nl], lhsT,
                                 fcw_sb[:, kr * PS + n0:kr * PS + n0 + nl],
                                 start=False, stop=False)

        ag3_in = dr.tile([BS, EMB], F32)
        ag3_out = dr.tile([B, EMB], F32, addr_space="Shared")
        nc.sync.dma_start(out=ag3_in[:], in_=ctx_sb[:])
        nc.gpsimd.collective_compute(
            "AllGather", ALU.bypass, replica_groups=[list(range(NCORES))],
            ins=[ag3_in[:]], outs=[ag3_out[:]])
        ctxf = sb.tile([B, EMB], F32)
        nc.sync.dma_start(out=ctxf[:], in_=ag3_out[:])

        ctxT = sb.tile([128, 2 * B], BF16)
        for k in range(2):
            tpc = pp.tile([128, B], F32, tag="p", name="tpc")
            nc.tensor.transpose(out=tpc[:], in_=ctxf[:, k * 128:(k + 1) * 128],
                                identity=ident[:B, :B])
            nc.vector.tensor_copy(ctxT[:, k * B:(k + 1) * B], tpc[:])

        # finish fc with the two ctx chunks (fc rows 0:256), evac, store
        for (n0, nl), ps_fc in zip(FC_CH, ps_fcs):
            for k in range(2):
                nc.tensor.matmul(ps_fc[:, :nl], ctxT[:, k * B:(k + 1) * B],
                                 fcw_sb[:, k * PS + n0:k * PS + n0 + nl],
                                 start=False, stop=(k == 1))
            lg = sb.tile([B, 512], F32, name="lg")
            nc.vector.tensor_copy(lg[:, :nl], ps_fc[:, :nl])
            nc.sync.dma_start(out=logits_s[:, n0:n0 + nl], in_=lg[:, :nl])

    nc.compile()
    return nc


def _prep_inputs(inputs):
    """Host-side sharding / layout prep (data movement + dtype casts only)."""
    f = lambda a: np.ascontiguousarray(np.asarray(a), dtype=np.float32)
    emb = f(inputs["poi_embedding"])
    query = f(inputs["query"])
    dec_h = f(inputs["dec_hidden"])
    cat_h = f(inputs["cat_dec_hidden"])[0]
    gk = f(inputs["gru_kernel"])
    gr = f(inputs["gru_rec_kernel"])
    gb = f(inputs["gru_bias"])
    w1 = f(inputs["W1_w"]); w1b = f(inputs["W1_b"]).reshape(U, 1)
    w2 = f(inputs["W2_w"]); w2b = f(inputs["W2_b"]).reshape(U, 1)
    vw = f(inputs["V_w"]).reshape(U, 1)
    fcw = f(inputs["fc_w"]); fcb = f(inputs["fc_b"]).reshape(1, POI)
    x = np.asarray(inputs["x"]).astype(np.int32).reshape(B, 1)

    emb_bf = emb.astype(ml_dtypes.bfloat16)
    embT = np.ascontiguousarray(emb.T)
    queryT = np.ascontiguousarray(query.T)
    dec_hT = np.ascontiguousarray(dec_h.T)
    cat_hT_bf = np.ascontiguousarray(cat_h.T).astype(ml_dtypes.bfloat16)

    in_maps = []
    for c in range(NCORES):
        cols = np.r_[c * US:(c + 1) * US,
                     U + c * US:U + (c + 1) * US,
                     2 * U + c * US:2 * U + (c + 1) * US]
        selm = np.zeros((B, BS), np.float32)
        selm[c * BS + np.arange(BS), np.arange(BS)] = 1.0
        in_maps.append({
            "emb_full": emb,
            "emb_bf": emb_bf,
            "embT_f": embT,
            "x_idx": x,
            "queryT": queryT,
            "dec_hT": dec_hT,
            "dec_hT_s": np.ascontiguousarray(dec_hT[c * US:(c + 1) * US, :]),
            "cat_hT_bf": cat_hT_bf,
            "gru_kT_s": np.ascontiguousarray(gk[:, cols]),
            "gru_rT_s": np.ascontiguousarray(gr[:, cols]),
            "gru_b0": np.ascontiguousarray(gb[0, cols]).reshape(3 * US, 1),
            "gru_b1": np.ascontiguousarray(gb[1, cols]).reshape(3 * US, 1),
            "w1": w1, "w1b": w1b, "w2": w2, "w2b": w2b, "vw": vw,
            "fcw_s_bf": np.ascontiguousarray(fcw[:, c * PS:(c + 1) * PS]).astype(ml_dtypes.bfloat16),
            "fcb_s": np.ascontiguousarray(fcb[:, c * PS:(c + 1) * PS]),
            "sel": selm,
        })
    return in_maps


def kernel(**inputs):
    if "nc" not in _CACHE:
        _CACHE["nc"] = _build()
    nc = _CACHE["nc"]
    in_maps = _prep_inputs(inputs)
    res = run_bass_kernel_spmd(nc, in_maps, list(range(NCORES)))
    r = res.results
    logits = np.concatenate([r[c]["logits_s"] for c in range(NCORES)], axis=1)
    h_new = r[0]["h_out"]
    return (logits, h_new, h_new.copy())


# revision 12
# speedup vs baseline: 1.3533x; 1.2639x over previous
"""Trainium2 Bass kernel for nn_Decoder (GRU + Bahdanau attention + fc decoder).

Reference computation (B=64, POI=5000, EMB=256, UNITS=512, QDIM=256):
    x1       = concat(emb[x], query)                   [B, 512]
    output_  = GRUCell(x1, dec_hidden)                 [B, 512]   (keras, reset_after)
    v_proj   = emb @ W1_w + W1_b                       [POI, 512]
    q_proj   = output_ @ W2_w + W2_b                   [B, 512]
    score    = tanh(v_proj[None] + q_proj[:,None]) @ V_w (+V_b)   [B, POI]
    attn     = softmax(score, axis=1)
    context  = sum(attn * emb, axis=1)                 [B, EMB]
    logits   = concat(context, output_, cat_dec_hidden[0]) @ fc_w + fc_b
    returns (logits, state, output_)  with state == output_ == GRU h_new

Sharding over 8 cores:
  - GRU: column-sharded over UNITS (64 cols/core, z/r/h gate-aligned), then
    AllGather of h_new^T (tiny, [64,64] -> [512,64]).
  - v_proj^T: computed REPLICATED on every core (PE is idle during the DMA
    front; avoids a 27us AllGather of the result).
  - attention tanh/score/softmax/context: BATCH-sharded (8 rows of B per
    core; selected via a one-hot `sel` input so the SPMD program is
    rank-agnostic). Softmax normalizer stays core-local (full POI per row).
  - context: AllGather [8,256] -> [64,256].
  - fc: POI-column-sharded ([1280, 625] per core); host concatenates logits.

V_b is omitted: softmax is shift-invariant so it cannot affect any output.
A_hat is unused by the reference.

Numerics: fp32 except (a) v_proj^T storage + tanh outputs (bf16, feeding the
score dot with bf16 V_w), (b) exp-scores + emb on the context matmul (bf16),
(c) fc weights and its lhsT operand (bf16). PSUM accumulation, GRU, softmax
normalizer, biases and outputs are fp32.
"""
import os
import sys

sys.path.insert(0, "/opt/trn_rl_repo")
os.environ.setdefault("MYCRO_LOCAL_CACHE", "1")

from contextlib import ExitStack

import numpy as np
import ml_dtypes

import concourse.bass as bass
import concourse.tile as tile
from concourse import bacc, mybir
from concourse.bass_utils import run_bass_kernel_spmd
from concourse.masks import make_identity

F32 = mybir.dt.float32
BF16 = mybir.dt.bfloat16
I32 = mybir.dt.int32
AF = mybir.ActivationFunctionType
ALU = mybir.AluOpType

NCORES = 8
POI, EMB, U, QDIM, B = 5000, 256, 512, 256, 64
PS = POI // NCORES          # 625   poi shard (fc columns)
BS = B // NCORES            # 8     batch shard
US = U // NCORES            # 64    units shard (GRU columns)
KIN = EMB + QDIM            # 512   GRU input dim
FCK = EMB + 2 * U           # 1280  fc contraction dim
PHL = POI // 2              # 2500  p-half (exp / transpose granularity)
# N-chunking of the full POI row for score matmuls (PSUM bank = 512 fp32)
SC_CH = [(j * 512, 512) for j in range(9)] + [(4608, POI - 4608)]
# N-chunking of the fc output shard
FC_CH = [(0, 512), (512, PS - 512)]

_CACHE = {}


def _build():
    nc = bacc.Bacc("TRN2", target_bir_lowering=False, debug=False,
                   num_devices=NCORES)

    def din(name, shape, dt=F32):
        return nc.dram_tensor(name, shape, dt, kind="ExternalInput").ap()

    emb_full = din("emb_full", [POI, EMB])          # gather source (fp32)
    emb_bf = din("emb_bf", [POI, EMB], BF16)        # context matmul rhs
    embT_f = din("embT_f", [EMB, POI], BF16)              # emb^T, for v_proj rhs
    x_idx = din("x_idx", [B, 1], I32)
    queryT = din("queryT", [QDIM, B])
    dec_hT = din("dec_hT", [U, B])
    dec_hT_s = din("dec_hT_s", [US, B])
    cat_hT_bf = din("cat_hT_bf", [U, B], BF16)
    gru_kT_s = din("gru_kT_s", [KIN, 3 * US])
    gru_rT_s = din("gru_rT_s", [U, 3 * US])
    gru_b0 = din("gru_b0", [3 * US, 1])
    gru_b1 = din("gru_b1", [3 * US, 1])
    w1 = din("w1", [EMB, U], BF16)
    w1b = din("w1b", [U, 1])
    w2 = din("w2", [U, U])
    w2b = din("w2b", [U, 1])
    vw = din("vw", [U, 1])
    fcw_s_bf = din("fcw_s_bf", [FCK, PS], BF16)
    fcb_s = din("fcb_s", [1, PS])
    sel = din("sel", [B, BS])

    logits_s = nc.dram_tensor("logits_s", [B, PS], F32, kind="ExternalOutput").ap()
    h_out = nc.dram_tensor("h_out", [B, U], F32, kind="ExternalOutput").ap()

    with tile.TileContext(nc) as tc, ExitStack() as ctx:
        sb = ctx.enter_context(tc.tile_pool(name="sb", bufs=1))
        tp = ctx.enter_context(tc.tile_pool(name="tpool", bufs=6))
        pp = ctx.enter_context(tc.tile_pool(name="pp", bufs=6, space="PSUM"))
        dr = ctx.enter_context(tc.tile_pool(name="dr", bufs=1, space="DRAM"))

        # ---- constants / ACT table prime ---------------------------------
        ident = sb.tile([128, 128], F32)
        make_identity(nc, ident[:])
        prime = sb.tile([1, 8], F32)
        nc.vector.memset(prime[:], 0.0)
        prime2 = sb.tile([1, 8], F32)
        # force the exp_and_others ACT table (has exp AND tanh) to load early
        nc.scalar.activation(prime2[:], prime[:], AF.Exp)
        ones1 = sb.tile([1, B], F32)
        nc.vector.memset(ones1[:], 1.0)

        # ---- input DMAs, critical-path order, one DMA per tensor ---------
        def load_chunked(name, src, rows, cols, dt=F32, q=None):
            """DRAM [rows, cols] -> SBUF [128, (rows/128)*cols] in one DMA."""
            nch = rows // 128
            t = sb.tile([128, nch * cols], dt, name=name)
            (q or nc.sync).dma_start(
                out=t[:].rearrange("p (k c) -> p k c", k=nch),
                in_=src.rearrange("(k p) c -> p k c", p=128))
            return t

        x_sb = sb.tile([B, 1], I32)
        nc.sync.dma_start(out=x_sb[:], in_=x_idx[:])
        b0_sb = sb.tile([US, 3], F32)
        nc.sync.dma_start(out=b0_sb[:].rearrange("p (g o) -> p g o", g=3),
                          in_=gru_b0.rearrange("(g p) o -> p g o", p=US))
        b1_sb = sb.tile([US, 3], F32)
        nc.sync.dma_start(out=b1_sb[:].rearrange("p (g o) -> p g o", g=3),
                          in_=gru_b1.rearrange("(g p) o -> p g o", p=US))
        dhTs_sb = sb.tile([US, B], F32)
        nc.sync.dma_start(out=dhTs_sb[:], in_=dec_hT_s[:])
        sel_sb = sb.tile([B, BS], F32)
        nc.sync.dma_start(out=sel_sb[:], in_=sel[:])
        gk_sb = load_chunked("gk_sb", gru_kT_s, KIN, 3 * US)
        gr_sb = load_chunked("gr_sb", gru_rT_s, U, 3 * US)
        qT_sb = load_chunked("qT_sb", queryT, QDIM, B)
        dhT_sb = load_chunked("dhT_sb", dec_hT, U, B)
        w1_sb = load_chunked("w1_sb", w1, EMB, U, dt=BF16)
        # embT streamed per 512-col block for v_proj (both k-chunks per DMA)
        et_tiles = []
        for (n0, nl) in SC_CH:
            et = tp.tile([128, 2 * 512], BF16, name="et", bufs=3)
            nc.sync.dma_start(
                out=et[:, :2 * nl].rearrange("p (k c) -> p k c", k=2),
                in_=embT_f[:, n0:n0 + nl].rearrange("(k p) c -> p k c", p=128))
            et_tiles.append(et)
        # ---- GRU (units-column shard, transposed layout [u_shard, b]) ----
        embx = sb.tile([B, EMB], F32)
        nc.gpsimd.indirect_dma_start(
            out=embx[:], out_offset=None, in_=emb_full[:],
            in_offset=bass.IndirectOffsetOnAxis(ap=x_sb[:, 0:1], axis=0))

        x1T_sb = sb.tile([128, 4 * B], F32)  # x1^T = [emb[x];query]^T, 4 k-chunks
        for c in range(2):
            tpx = pp.tile([128, B], F32, tag="p", name="tpx")
            nc.tensor.transpose(out=tpx[:], in_=embx[:, c * 128:(c + 1) * 128],
                                identity=ident[:B, :B])
            nc.vector.tensor_copy(x1T_sb[:, c * B:(c + 1) * B], tpx[:])
        nc.vector.tensor_copy(x1T_sb[:, 2 * B:4 * B], qT_sb[:, 0:2 * B])

        def gate_slice(t, k, g):
            base = k * 3 * US + g * US
            return t[:, base:base + US]

        def gate_psum(g, rec_too):
            ps = pp.tile([US, B], F32, tag="p", name=f"g{g}")
            nmm = 8 if rec_too else 4
            i = 0
            for k in range(4):  # kernel part, K chunks of x1T
                nc.tensor.matmul(ps[:], gate_slice(gk_sb, k, g),
                                 x1T_sb[:, k * B:(k + 1) * B],
                                 start=(i == 0), stop=(i == nmm - 1))
                i += 1
            if rec_too:
                for k in range(4):
                    nc.tensor.matmul(ps[:], gate_slice(gr_sb, k, g),
                                     dhT_sb[:, k * B:(k + 1) * B],
                                     start=False, stop=(i == nmm - 1))
                    i += 1
            return ps

        ps_z = gate_psum(0, True)
        ps_r = gate_psum(1, True)
        ps_xh = gate_psum(2, False)
        ps_hh = pp.tile([US, B], F32, tag="p", name="ps_hh")
        for k in range(4):
            nc.tensor.matmul(ps_hh[:], gate_slice(gr_sb, k, 2),
                             dhT_sb[:, k * B:(k + 1) * B], start=(k == 0), stop=(k == 3))

        # combined half-biases for z, r (sigmoid(x) = 0.5 + 0.5*tanh(x/2))
        bzr = sb.tile([US, 2], F32)
        nc.vector.tensor_add(bzr[:], b0_sb[:, 0:2], b1_sb[:, 0:2])
        bzr_h = sb.tile([US, 2], F32)
        nc.vector.tensor_scalar_mul(bzr_h[:], bzr[:], 0.5)

        zt = sb.tile([US, B], F32)
        nc.scalar.activation(zt[:], ps_z[:], AF.Tanh, bias=bzr_h[:, 0:1], scale=0.5)
        rt = sb.tile([US, B], F32)
        nc.scalar.activation(rt[:], ps_r[:], AF.Tanh, bias=bzr_h[:, 1:2], scale=0.5)

        hh = sb.tile([US, B], F32)
        nc.scalar.add(hh[:], ps_hh[:], b1_sb[:, 2:3])
        m1 = sb.tile([US, B], F32)
        nc.vector.tensor_mul(m1[:], rt[:], hh[:])
        m2 = sb.tile([US, B], F32)
        nc.vector.tensor_add(m2[:], hh[:], m1[:])
        # c_pre = xh + 0.5*(hh + rt*hh)   (r*hh with r = 0.5+0.5*rt)
        cpre = sb.tile([US, B], F32)
        nc.vector.scalar_tensor_tensor(cpre[:], in0=m2[:], scalar=0.5,
                                       in1=ps_xh[:], op0=ALU.mult, op1=ALU.add)
        ct = sb.tile([US, B], F32)
        nc.scalar.activation(ct[:], cpre[:], AF.Tanh, bias=b0_sb[:, 2:3], scale=1.0)
        # h_new = 0.5*((h_prev + c) + zt*(h_prev - c))
        s_ = sb.tile([US, B], F32)
        nc.vector.tensor_add(s_[:], dhTs_sb[:], ct[:])
        d_ = sb.tile([US, B], F32)
        nc.vector.tensor_sub(d_[:], dhTs_sb[:], ct[:])
        m_ = sb.tile([US, B], F32)
        nc.vector.tensor_mul(m_[:], zt[:], d_[:])
        hn2 = sb.tile([US, B], F32)
        nc.vector.tensor_add(hn2[:], s_[:], m_[:])
        hnT_s = sb.tile([US, B], F32)
        nc.vector.tensor_scalar_mul(hnT_s[:], hn2[:], 0.5)

        # AllGather h_new^T shards -> full h^T [512, 64]
        ag1_in = dr.tile([US, B], F32)
        ag1_out = dr.tile([U, B], F32, addr_space="Shared")
        nc.sync.dma_start(out=ag1_in[:], in_=hnT_s[:])
        nc.gpsimd.collective_compute(
            "AllGather", ALU.bypass, replica_groups=[list(range(NCORES))],
            ins=[ag1_in[:]], outs=[ag1_out[:]])
        w2_sb = load_chunked("w2_sb", w2, U, U)
        wbv_sb = sb.tile([128, 12], F32)
        for i, t in enumerate((w1b, w2b, vw)):
            nc.sync.dma_start(
                out=wbv_sb[:, 4 * i:4 * (i + 1)].rearrange("p (k o) -> p k o", k=4),
                in_=t.rearrange("(k p) o -> p k o", p=128))
        w1b_sb, w2b_sb, vw_sb = wbv_sb[:, 0:4], wbv_sb[:, 4:8], wbv_sb[:, 8:12]
        vw_bf = sb.tile([128, 4], BF16)
        nc.vector.tensor_copy(vw_bf[:], vw_sb)
        chT_sb = load_chunked("chT_sb", cat_hT_bf, U, B, dt=BF16)
        fcb_sb = sb.tile([1, PS], F32)
        nc.sync.dma_start(out=fcb_sb[:], in_=fcb_s[:])
        fcw_sb = load_chunked("fcw_sb", fcw_s_bf, FCK, PS, dt=BF16)
        emb_sb = load_chunked("emb_sb", emb_bf[0:POI - 8, :], POI - 8, EMB, dt=BF16)
        emb_tail = sb.tile([8, EMB], BF16)
        nc.sync.dma_start(out=emb_tail[:], in_=emb_bf[POI - 8:POI, :])

        # ---- v_proj^T computed fully on every core (bf16 result) ---------
        vfull = sb.tile([128, 4 * POI], BF16)
        for j, (n0, nl) in enumerate(SC_CH):
            for m in range(4):
                ps_v = pp.tile([128, 512], F32, tag="p", name="ps_v")
                for k in range(2):
                    nc.tensor.matmul(ps_v[:, :nl],
                                     w1_sb[:, k * U + m * 128:k * U + (m + 1) * 128],
                                     et_tiles[j][:, k * nl:(k + 1) * nl],
                                     start=(k == 0), stop=(k == 1))
                nc.vector.tensor_copy(vfull[:, m * POI + n0:m * POI + n0 + nl],
                                      ps_v[:, :nl])

        hT_sb = sb.tile([128, 4 * B], F32)
        nc.sync.dma_start(out=hT_sb[:].rearrange("p (k b) -> p k b", k=4),
                          in_=ag1_out.rearrange("(k p) b -> p k b", p=128))
        hT_bf = sb.tile([128, 4 * B], BF16)   # bf16 copy for the fc lhsT
        nc.vector.tensor_copy(hT_bf[:], hT_sb[:])

        # h (non-transposed) for the state output + sel matmul
        h_sb = sb.tile([B, U], F32)
        for k in range(4):
            tph = pp.tile([B, 128], F32, tag="p", name="tph")
            nc.tensor.transpose(out=tph[:], in_=hT_sb[:, k * B:(k + 1) * B],
                                identity=ident[:, :])
            nc.vector.tensor_copy(h_sb[:, k * 128:(k + 1) * 128], tph[:])
        nc.sync.dma_start(out=h_out[:], in_=h_sb[:])

        # my 8 batch rows of h via one-hot selection (keeps program rank-agnostic)
        ps_hm = pp.tile([BS, U], F32, tag="p", name="ps_hm")
        nc.tensor.matmul(ps_hm[:], sel_sb[:], h_sb[:], start=True, stop=True)
        hm_sb = sb.tile([BS, U], F32)
        nc.vector.tensor_copy(hm_sb[:], ps_hm[:])
        hTm_sb = sb.tile([128, 4 * BS], F32)
        for k in range(4):
            tpm = pp.tile([128, BS], F32, tag="p", name="tpm")
            nc.tensor.transpose(out=tpm[:], in_=hm_sb[:, k * 128:(k + 1) * 128],
                                identity=ident[:BS, :BS])
            nc.vector.tensor_copy(hTm_sb[:, k * BS:(k + 1) * BS], tpm[:])

        # biasT[u, j] = q_proj^T + W1_b + W2_b   for my 8 batch rows
        bv_sb = sb.tile([128, 4], F32)
        nc.vector.tensor_add(bv_sb[:], w1b_sb, w2b_sb)
        biasT = sb.tile([128, 4 * BS], F32)
        for m in range(4):
            ps_q = pp.tile([128, BS], F32, tag="p", name="ps_q")
            for k in range(4):
                nc.tensor.matmul(ps_q[:], w2_sb[:, k * U + m * 128:k * U + (m + 1) * 128],
                                 hTm_sb[:, k * BS:(k + 1) * BS],
                                 start=(k == 0), stop=(k == 3))
            nc.scalar.add(biasT[:, m * BS:(m + 1) * BS], ps_q[:], bv_sb[:, m:m + 1])

        # ---- attention main loop (ACT-bound) -----------------------------
        # tanh in [128, 5000] tiles: 4 ACT instructions per batch row.
        # Wave A: chunks 0-4 accumulate per-tanh (keeps PE fed).
        # Wave B: chunks 5-9 chunk-major after all 4 tanhs (long warm PE
        # burst, early PSUM evac keeps <= 3 score banks live).
        scores = sb.tile([BS, POI], F32)
        n_ph = sb.tile([BS, 2], F32)
        for bi in range(BS):
            t_tiles = []
            scs_a = [pp.tile([1, 512], F32, tag="p", name=f"sa{j}")
                     for j in range(5)]
            for m in range(4):
                t_bf = tp.tile([128, POI], BF16, tag="T", name="t_bf")
                t_tiles.append(t_bf)
                nc.scalar.activation(
                    t_bf[:], vfull[:, m * POI:(m + 1) * POI],
                    AF.Tanh, bias=biasT[:, m * BS + bi:m * BS + bi + 1])
                for j in range(5):
                    n0, nl = SC_CH[j]
                    nc.tensor.matmul(scs_a[j][:, :nl], vw_bf[:, m:m + 1],
                                     t_bf[:, n0:n0 + nl],
                                     start=(m == 0), stop=(m == 3))
            q = nc.sync if bi % 2 == 0 else nc.gpsimd
            for j in range(5):
                n0, nl = SC_CH[j]
                st = tp.tile([1, 512], F32, name="st", bufs=3)
                nc.vector.tensor_copy(st[:1, :nl], scs_a[j][:1, :nl])
                q.dma_start(out=scores[bi:bi + 1, n0:n0 + nl], in_=st[:1, :nl])
            for j in range(5, 10):
                n0, nl = SC_CH[j]
                ps_sc = pp.tile([1, 512], F32, tag="p", name="ps_sc")
                for m in range(4):
                    nc.tensor.matmul(ps_sc[:, :nl], vw_bf[:, m:m + 1],
                                     t_tiles[m][:, n0:n0 + nl],
                                     start=(m == 0), stop=(m == 3))
                st = tp.tile([1, 512], F32, name="st", bufs=3)
                nc.vector.tensor_copy(st[:1, :nl], ps_sc[:1, :nl])
                q.dma_start(out=scores[bi:bi + 1, n0:n0 + nl], in_=st[:1, :nl])

        # exp in place per p-half; accumulate the softmax normalizer
        expT = sb.tile([128, 40 * BS], BF16)
        for ph in range(2):
            nc.scalar.activation(scores[:, ph * PHL:(ph + 1) * PHL],
                                 scores[:, ph * PHL:(ph + 1) * PHL],
                                 AF.Exp, accum_out=n_ph[:, ph:ph + 1])
            # transpose this half's exp scores into [128, 8] chunks (bf16)
            for c in range(20 * ph, 20 * (ph + 1)):
                w = 128 if c < 39 else POI - 39 * 128  # 8-wide tail chunk
                tpe = pp.tile([128, BS], F32, tag="p", name="tpe")
                nc.tensor.transpose(out=tpe[:w, :],
                                    in_=scores[:, c * 128:c * 128 + w],
                                    identity=ident[:BS, :BS])
                nc.vector.tensor_copy(expT[:w, c * BS:(c + 1) * BS], tpe[:w, :])

        n_sb = sb.tile([BS, 1], F32)
        nc.vector.tensor_add(n_sb[:], n_ph[:, 0:1], n_ph[:, 1:2])
        rn_sb = sb.tile([BS, 1], F32)
        nc.vector.reciprocal(rn_sb[:], n_sb[:])

        # context (unnormalized): ctx[j, e] = sum_p exp[j, p] * emb[p, e]
        ps_ctx = pp.tile([BS, EMB], F32, tag="p", name="ps_ctx")
        for c in range(39):
            nc.tensor.matmul(ps_ctx[:], expT[:, c * BS:(c + 1) * BS],
                             emb_sb[:, c * EMB:(c + 1) * EMB],
                             start=(c == 0), stop=False)
        nc.tensor.matmul(ps_ctx[:], expT[:8, 39 * BS:40 * BS], emb_tail[:],
                         start=False, stop=True)
        ctx_sb = sb.tile([BS, EMB], F32)
        nc.vector.tensor_scalar_mul(ctx_sb[:], ps_ctx[:], rn_sb[:, 0:1])

        # ---- fc partials (bias + h + cat chunks) BEFORE the ctx AllGather,
        # so these matmuls execute on PE while the collective runs.
        ps_fcs = []
        for (n0, nl) in FC_CH:
            ps_fc = pp.tile([B, 512], F32, tag="fc", name="ps_fc", bufs=2)
            ps_fcs.append(ps_fc)
            nc.tensor.matmul(ps_fc[:, :nl], ones1[:, :], fcb_sb[:, n0:n0 + nl],
                             start=True, stop=False)
            for i in range(8):   # h chunks (fc rows 256:768), cat (768:1280)
                kr = 2 + i
                src = hT_bf if i < 4 else chT_sb
                lhsT = src[:, (i % 4) * B:(i % 4 + 1) * B]
                nc.tensor.matmul(ps_fc[:, :nl], lhsT,
                                 fcw_sb[:, kr * PS + n0:kr * PS + n0 + nl],
                                 start=False, stop=False)

        ag3_in = dr.tile([BS, EMB], F32)
        ag3_out = dr.tile([B, EMB], F32, addr_space="Shared")
        nc.sync.dma_start(out=ag3_in[:], in_=ctx_sb[:])
        nc.gpsimd.collective_compute(
            "AllGather", ALU.bypass, replica_groups=[list(range(NCORES))],
            ins=[ag3_in[:]], outs=[ag3_out[:]])
        ctxf = sb.tile([B, EMB], F32)
        nc.sync.dma_start(out=ctxf[:], in_=ag3_out[:])

        ctxT = sb.tile([128, 2 * B], BF16)
        for k in range(2):
            tpc = pp.tile([128, B], F32, tag="p", name="tpc")
            nc.tensor.transpose(out=tpc[:], in_=ctxf[:, k * 128:(k + 1) * 128],
                                identity=ident[:B, :B])
            nc.vector.tensor_copy(ctxT[:, k * B:(k + 1) * B], tpc[:])

        # finish fc with the two ctx chunks (fc rows 0:256), evac, store
        for (n0, nl), ps_fc in zip(FC_CH, ps_fcs):
            for k in range(2):
                nc.tensor.matmul(ps_fc[:, :nl], ctxT[:, k * B:(k + 1) * B],
                                 fcw_sb[:, k * PS + n0:k * PS + n0 + nl],
                                 start=False, stop=(k == 1))
            lg = sb.tile([B, 512], F32, name="lg")
            nc.vector.tensor_copy(lg[:, :nl], ps_fc[:, :nl])
            nc.sync.dma_start(out=logits_s[:, n0:n0 + nl], in_=lg[:, :nl])

    nc.compile()
    return nc


def _prep_inputs(inputs):
    """Host-side sharding / layout prep (data movement + dtype casts only)."""
    f = lambda a: np.ascontiguousarray(np.asarray(a), dtype=np.float32)
    emb = f(inputs["poi_embedding"])
    query = f(inputs["query"])
    dec_h = f(inputs["dec_hidden"])
    cat_h = f(inputs["cat_dec_hidden"])[0]
    gk = f(inputs["gru_kernel"])
    gr = f(inputs["gru_rec_kernel"])
    gb = f(inputs["gru_bias"])
    w1 = f(inputs["W1_w"]); w1b = f(inputs["W1_b"]).reshape(U, 1)
    w2 = f(inputs["W2_w"]); w2b = f(inputs["W2_b"]).reshape(U, 1)
    vw = f(inputs["V_w"]).reshape(U, 1)
    fcw = f(inputs["fc_w"]); fcb = f(inputs["fc_b"]).reshape(1, POI)
    x = np.asarray(inputs["x"]).astype(np.int32).reshape(B, 1)

    emb_bf = emb.astype(ml_dtypes.bfloat16)
    embT = np.ascontiguousarray(emb.T)
    queryT = np.ascontiguousarray(query.T)
    dec_hT = np.ascontiguousarray(dec_h.T)
    cat_hT_bf = np.ascontiguousarray(cat_h.T).astype(ml_dtypes.bfloat16)

    in_maps = []
    for c in range(NCORES):
        cols = np.r_[c * US:(c + 1) * US,
                     U + c * US:U + (c + 1) * US,
                     2 * U + c * US:2 * U + (c + 1) * US]
        selm = np.zeros((B, BS), np.float32)
        selm[c * BS + np.arange(BS), np.arange(BS)] = 1.0
        in_maps.append({
            "emb_full": emb,
            "emb_bf": emb_bf,
            "embT_f": embT.astype(ml_dtypes.bfloat16),
            "x_idx": x,
            "queryT": queryT,
            "dec_hT": dec_hT,
            "dec_hT_s": np.ascontiguousarray(dec_hT[c * US:(c + 1) * US, :]),
            "cat_hT_bf": cat_hT_bf,
            "gru_kT_s": np.ascontiguousarray(gk[:, cols]),
            "gru_rT_s": np.ascontiguousarray(gr[:, cols]),
            "gru_b0": np.ascontiguousarray(gb[0, cols]).reshape(3 * US, 1),
            "gru_b1": np.ascontiguousarray(gb[1, cols]).reshape(3 * US, 1),
            "w1": w1.astype(ml_dtypes.bfloat16), "w1b": w1b, "w2": w2, "w2b": w2b, "vw": vw,
            "fcw_s_bf": np.ascontiguousarray(fcw[:, c * PS:(c + 1) * PS]).astype(ml_dtypes.bfloat16),
            "fcb_s": np.ascontiguousarray(fcb[:, c * PS:(c + 1) * PS]),
            "sel": selm,
        })
    return in_maps


def kernel(**inputs):
    if "nc" not in _CACHE:
        _CACHE["nc"] = _build()
    nc = _CACHE["nc"]
    in_maps = _prep_inputs(inputs)
    res = run_bass_kernel_spmd(nc, in_maps, list(range(NCORES)))
    r = res.results
    logits = np.concatenate([r[c]["logits_s"] for c in range(NCORES)], axis=1)
    h_new = r[0]["h_out"]
    return (logits, h_new, h_new.copy())
